# revision 1
# baseline (speedup 1.0000x reference)
"""Trainium2 Bass kernel for CausalMessagePassingLayer (2x GCN + gated scatter).

Sharding: 8 cores = 4 samples x 2 halves of the embedding dim (D=768 -> 384).
Each core is fully independent (no collectives).

v2 pipeline (fp8 DoubleRow matmuls, host-permuted layouts):
  - host pre-permutes t[t2x] into transposed fp8 xT tiles (indices are
    known at compile time, so the x "gather" is pure input marshalling)
  - xw: y = dinv_src * (x @ W[:, half]) via fp8 DoubleRow matmuls, y kept
    in SBUF as fp8
  - GCN aggregation z[dst] += count * y[src] as dense per-tile adjacency
    blocks (fp8 counts, incl. self-loops) with DoubleRow pairs of src
    tiles accumulating in PSUM
  - e = z * (dinv_dst * tanh(gate)) + tanh(gate)*bias -> bf16 in SBUF
  - final combine runs in edges-row order: tokens are processed as
    [4096 tokens of the edges image (e_edges is a contiguous SBUF
    operand), then the nodes-only tokens ordered by nodes-row]; only
    e_nodes rows are gathered (from a DRAM copy), tokens in neither
    image are exact f32 t passthrough assembled on the host
  - out rows written f32, host inverse-permutes to [B, S, D]

Host-side work is restricted to index/descriptor preparation (dense count
blocks, degree counts, inverse permutations, index wrapping for the DGE)
and dtype/layout marshalling of inputs; all floating-point math runs on
device.
"""

import numpy as np
import ml_dtypes

B, S, D, N, E = 4, 8192, 768, 4096, 32768
H = D // 2            # per-core half of embedding dim
P = 128
NT = N // P           # 32 node tiles per graph
KT = D // P           # 6 k-tiles of the contraction dim
L2 = 2560             # padded count of nodes-only tokens (actual ~2048)
C = NT + L2 // P      # 52 row-chunks of final output per core
FINW = 4              # chunks per final wave

bf16 = ml_dtypes.bfloat16
f8 = ml_dtypes.float8_e4m3

# test-harness knobs (the grading harness just calls kernel() and these stay default)
TRACE = False
TRACE_CORES = None
LAST_RESULT = None


def _wrap_idx(idx):
    """dma_gather index layout: i -> [i % 16, i // 16], replicated to 128 partitions."""
    n = idx.shape[0]
    assert n % 16 == 0
    w = idx.astype(np.int16).reshape(n // 16, 16).T
    return np.ascontiguousarray(np.tile(w, (8, 1)))


def _prep_graph(ei):
    """Dense adjacency-count blocks (incl. self loops) + degree counts.

    Returns (A_blocks, deg): A_blocks[t, p, sc, q] = #edges src=sc*128+p ->
    dst=t*128+q, laid out so A_blocks[t] is directly the stack of matmul
    lhsT tiles for dst-tile t. deg includes the self loop.
    """
    s = np.concatenate([ei[0].astype(np.int64), np.arange(N, dtype=np.int64)])
    d = np.concatenate([ei[1].astype(np.int64), np.arange(N, dtype=np.int64)])
    A = np.zeros((N, N), np.float32)
    np.add.at(A, (d, s), 1.0)
    deg = np.bincount(d, minlength=N).astype(np.int32)
    Ab = np.ascontiguousarray(
        A.reshape(NT, P, NT, P).transpose(0, 3, 2, 1)
    ).astype(f8)
    return Ab, deg


def _xT(x):
    """[N, D] -> bf16 [P, KT, N] with K element d = s*128 + p."""
    return np.ascontiguousarray(x.T.reshape(KT, P, N).transpose(1, 0, 2)).astype(bf16)


def kernel(**inputs):
    import concourse.bacc as bacc
    import concourse.mybir as mybir
    import concourse.tile as tile
    from concourse.bass_utils import run_bass_kernel_spmd

    f32, bft, fp8, i16, i32 = (
        mybir.dt.float32,
        mybir.dt.bfloat16,
        mybir.dt.float8e4,
        mybir.dt.int16,
        mybir.dt.int32,
    )
    DR = mybir.MatmulPerfMode.DoubleRow

    t_full = np.asarray(inputs["token_embeddings"], dtype=np.float32)
    W = {
        "e": np.asarray(inputs["W_edges"], dtype=np.float32),
        "n": np.asarray(inputs["W_nodes"], dtype=np.float32),
    }
    bias = {
        "e": np.asarray(inputs["b_edges"], dtype=np.float32),
        "n": np.asarray(inputs["b_nodes"], dtype=np.float32),
    }
    gate = {
        "e": np.asarray(inputs["gate_a"], dtype=np.float32).reshape(1, 1),
        "n": np.asarray(inputs["gate_b"], dtype=np.float32).reshape(1, 1),
    }
    t2x = {
        "e": np.asarray(inputs["tokens2edges"], dtype=np.int64),
        "n": np.asarray(inputs["tokens2nodes"], dtype=np.int64),
    }
    x2t = {
        "e": np.asarray(inputs["edges2tokens"], dtype=np.int64),
        "n": np.asarray(inputs["nodes2tokens"], dtype=np.int64),
    }
    ei = {
        "e": np.asarray(inputs["edge_index_edges"], dtype=np.int64),
        "n": np.asarray(inputs["edge_index_nodes"], dtype=np.int64),
    }

    gcns = ("n", "e")  # nodes first so e_nodes DRAM write overlaps the edges GCN

    # ---- per-sample host index prep ----
    samp = []
    for b in range(B):
        sd = {}
        for g in ("e", "n"):
            sd[f"A_{g}"], sd[f"deg_{g}"] = _prep_graph(ei[g][b])
        pos_n = np.full(S, N, np.int64)
        pos_n[x2t["n"][b]] = np.arange(N)
        e_img = x2t["e"][b]                      # token of edges-row i
        sd["idx1"] = pos_n[e_img]                # nodes-row for seg1 tokens
        in_e = np.zeros(S, bool)
        in_e[e_img] = True
        j2 = np.where(~in_e[x2t["n"][b]])[0]     # nodes-rows of nodes-only tokens
        assert len(j2) <= L2, len(j2)
        sd["len2"] = len(j2)
        sd["idx2"] = np.concatenate([j2, np.full(L2 - len(j2), N, np.int64)])
        sd["seg2_tok"] = x2t["n"][b][j2]
        in_n = np.zeros(S, bool)
        in_n[x2t["n"][b]] = True
        sd["seg3_tok"] = np.where(~in_e & ~in_n)[0]
        # t rows in final processing order [4096 e-image | nodes-only | pad]
        rows = np.zeros((C * P, D), np.float32)
        rows[:N] = t_full[b][e_img]
        rows[N : N + len(j2)] = t_full[b][sd["seg2_tok"]]
        sd["t_rows"] = rows
        samp.append(sd)

    # ---- per-core host data ----
    core_data = []
    for b in range(B):
        sd = samp[b]
        for h in range(2):
            d = {}
            for g in gcns:
                d[f"xT_{g}"] = _xT(t_full[b][t2x[g][b]])
                d[f"W_{g}"] = np.ascontiguousarray(
                    W[g][:, h * H : (h + 1) * H].reshape(KT, P, H).transpose(1, 0, 2)
                ).astype(bf16)
                d[f"bias_{g}"] = np.ascontiguousarray(bias[g][None, h * H : (h + 1) * H])
                d[f"gate_{g}"] = gate[g]
                d[f"A_{g}"] = sd[f"A_{g}"]
                d[f"deg_pc_{g}"] = np.ascontiguousarray(
                    sd[f"deg_{g}"].reshape(NT, P).T
                )
            d["idx1_w"] = _wrap_idx(sd["idx1"])
            d["idx2_w"] = _wrap_idx(sd["idx2"])
            d["t_seg"] = np.ascontiguousarray(
                sd["t_rows"].reshape(C, P, D).transpose(1, 0, 2)[:, :, h * H : (h + 1) * H]
            ).astype(bf16)
            core_data.append(d)

    # ---- build the SPMD program ----
    nc = bacc.Bacc("TRN2", target_bir_lowering=False, debug=False, num_swdge_queues=1)

    ins_d = {}
    for g in gcns:
        ins_d[f"xT_{g}"] = nc.declare_dram_parameter(f"xT_{g}", [P, KT, N], bft, isOutput=False)
        ins_d[f"W_{g}"] = nc.declare_dram_parameter(f"W_{g}", [P, KT, H], bft, isOutput=False)
        ins_d[f"bias_{g}"] = nc.declare_dram_parameter(f"bias_{g}", [1, H], f32, isOutput=False)
        ins_d[f"gate_{g}"] = nc.declare_dram_parameter(f"gate_{g}", [1, 1], f32, isOutput=False)
        ins_d[f"A_{g}"] = nc.declare_dram_parameter(
            f"A_{g}", [NT, P, NT, P], fp8, isOutput=False
        )
        ins_d[f"deg_pc_{g}"] = nc.declare_dram_parameter(
            f"deg_pc_{g}", [P, NT], i32, isOutput=False
        )
    ins_d["idx1_w"] = nc.declare_dram_parameter("idx1_w", [P, N // 16], i16, isOutput=False)
    ins_d["idx2_w"] = nc.declare_dram_parameter("idx2_w", [P, L2 // 16], i16, isOutput=False)
    ins_d["t_seg"] = nc.declare_dram_parameter("t_seg", [P, C, H], bft, isOutput=False)
    out_d = nc.declare_dram_parameter("out", [P, C, H], f32, isOutput=True)

    en_d = nc.dram_tensor("e_n_rows", [N + 1, H], bft)

    with tile.TileContext(nc) as tc:
        with (
            tc.tile_pool(name="cst", bufs=1) as cst,
            tc.tile_pool(name="idxp", bufs=1) as idxp,
            tc.tile_pool(name="xt", bufs=2) as xtp,
            tc.tile_pool(name="yp", bufs=1) as yp,
            tc.tile_pool(name="ap", bufs=2) as apool,
            tc.tile_pool(name="ep", bufs=1) as ep,
            tc.tile_pool(name="enp", bufs=1) as enp,
            tc.tile_pool(name="tp", bufs=1) as tp,
            tc.tile_pool(name="enw", bufs=2) as enw,
            tc.tile_pool(name="fout", bufs=1) as finp,
            tc.tile_pool(name="psxw", bufs=2, space="PSUM") as psxw,
            tc.tile_pool(name="psz", bufs=2, space="PSUM") as psz,
        ):
            # ---------- setup ----------
            zrow = cst.tile([1, H], bft)
            nc.vector.memset(zrow[:], 0)
            nc.sync.dma_start(out=en_d[N : N + 1, :], in_=zrow[:])

            Wsb, bias_ga, dinv, dinv_ga = {}, {}, {}, {}
            for g in gcns:
                Wsb[g] = cst.tile([P, KT, H], bft, name=f"W_{g}", tag=f"W_{g}")
                nc.sync.dma_start(out=Wsb[g][:], in_=ins_d[f"W_{g}"][:])

                gcol = cst.tile([P, 1], f32, name=f"gcol_{g}", tag=f"gcol_{g}")
                nc.sync.dma_start(
                    out=gcol[:], in_=ins_d[f"gate_{g}"][:1, :].to_broadcast([P, 1])
                )
                tanh_g = cst.tile([P, 1], f32, name=f"tanh_{g}", tag=f"tanh_{g}")
                nc.scalar.activation(
                    out=tanh_g[:], in_=gcol[:], func=mybir.ActivationFunctionType.Tanh
                )

                brow = cst.tile([P, H], f32, name=f"brow_{g}", tag=f"brow_{g}")
                nc.sync.dma_start(
                    out=brow[:], in_=ins_d[f"bias_{g}"][:1, :].to_broadcast([P, H])
                )
                bias_ga[g] = cst.tile([P, H], f32, name=f"biasga_{g}", tag=f"biasga_{g}")
                nc.vector.tensor_scalar_mul(bias_ga[g][:], brow[:], tanh_g[:, :1])

                deg_i = cst.tile([P, NT], i32, name=f"degi_{g}", tag=f"degi_{g}")
                nc.sync.dma_start(out=deg_i[:], in_=ins_d[f"deg_pc_{g}"][:])
                deg_f = cst.tile([P, NT], f32, name=f"degf_{g}", tag=f"degf_{g}")
                nc.vector.tensor_copy(out=deg_f[:], in_=deg_i[:])
                rdeg = cst.tile([P, NT], f32, name=f"rdeg_{g}", tag=f"rdeg_{g}")
                nc.vector.reciprocal(rdeg[:], deg_f[:])
                dinv[g] = cst.tile([P, NT], f32, name=f"dinv_{g}", tag=f"dinv_{g}")
                nc.scalar.sqrt(dinv[g][:], rdeg[:])
                dinv_ga[g] = cst.tile([P, NT], f32, name=f"dinvga_{g}", tag=f"dinvga_{g}")
                nc.vector.tensor_scalar_mul(dinv_ga[g][:], dinv[g][:], tanh_g[:, :1])

            idx1 = idxp.tile([P, N // 16], i16, name="idx1", tag="idx1")
            nc.sync.dma_start(out=idx1[:], in_=ins_d["idx1_w"][:])
            idx2 = idxp.tile([P, L2 // 16], i16, name="idx2", tag="idx2")
            nc.sync.dma_start(out=idx2[:], in_=ins_d["idx2_w"][:])

            en_all = enp.tile([P, C, H], bft, tag="en_all")
            e_sb = {}

            # ---------- per-GCN ----------
            for g in gcns:
                xT = xtp.tile([P, KT, N], bft, tag="xT")
                nc.sync.dma_start(out=xT[:], in_=ins_d[f"xT_{g}"][:])

                y_sb = yp.tile([P, NT, H], fp8, tag="ysb")
                for c in range(NT):
                    ps = psxw.tile([P, H], f32)
                    for j in range(KT):
                        nc.tensor.matmul(
                            out=ps[:],
                            lhsT=xT[:, j, c * P : (c + 1) * P],
                            rhs=Wsb[g][:, j, :],
                            start=(j == 0),
                            stop=(j == KT - 1),
                        )
                    nc.scalar.activation(
                        out=y_sb[:, c, :],
                        in_=ps[:],
                        func=mybir.ActivationFunctionType.Copy,
                        scale=dinv[g][:, c : c + 1],
                    )

                # aggregation via dense per-tile adjacency count blocks.
                # for the edges GCN, e rows land in per-wave tiles and the
                # final-combine wave is emitted as soon as its 4 dst tiles are
                # done, so vector/DMA overlap the remaining aggregation
                if g == "e":
                    e_sb[g] = [
                        ep.tile([P, FINW, H], bft, name=f"esb_e_{w}", tag=f"esb_e_{w}")
                        for w in range(NT // FINW)
                    ]
                for t_i in range(NT):
                    At = apool.tile([P, NT, P], fp8, name="At", tag="At")
                    nc.sync.dma_start(out=At[:], in_=ins_d[f"A_{g}"][t_i])
                    zt = psz.tile([P, H], f32, name="zt", tag="zt")
                    for j in range(NT // 2):
                        nc.tensor.matmul(
                            out=zt[:],
                            lhsT=At[:, 2 * j : 2 * j + 2, :],
                            rhs=y_sb[:, 2 * j : 2 * j + 2, :],
                            start=(j == 0),
                            stop=(j == NT // 2 - 1),
                            perf_mode=DR,
                        )
                    if g == "e":
                        e_dst = e_sb[g][t_i // FINW][:, t_i % FINW, :]
                    else:
                        en_row = enw.tile([P, H], bft, name="en_row", tag="en_row")
                        e_dst = en_row[:]
                    nc.vector.scalar_tensor_tensor(
                        out=e_dst,
                        in0=zt[:],
                        scalar=dinv_ga[g][:, t_i : t_i + 1],
                        in1=bias_ga[g][:],
                        op0=mybir.AluOpType.mult,
                        op1=mybir.AluOpType.add,
                    )
                    if g == "n":
                        # stream e_nodes rows to DRAM as they complete
                        nc.sync.dma_start(
                            out=en_d[t_i * P : (t_i + 1) * P, :], in_=en_row[:]
                        )
                if g == "n":
                    GW = 1024  # max indices per dma_gather the HW accepts
                    for k in range(N // GW):
                        nc.gpsimd.dma_gather(
                            out_ap=en_all[:, k * (GW // P) : (k + 1) * (GW // P), :],
                            in_ap=en_d[:],
                            idxs_ap=idx1[:, k * (GW // 16) : (k + 1) * (GW // 16)],
                            num_idxs=GW,
                            num_idxs_reg=GW,
                            elem_size=H,
                            queue_num=0,
                        )
                    for k in range(L2 // 512):
                        nc.gpsimd.dma_gather(
                            out_ap=en_all[:, NT + k * 4 : NT + (k + 1) * 4, :],
                            in_ap=en_d[:],
                            idxs_ap=idx2[:, k * 32 : (k + 1) * 32],
                            num_idxs=512,
                            num_idxs_reg=512,
                            elem_size=H,
                            queue_num=0,
                        )
                    # nodes-only final waves depend only on the gathers and t
                    # rows -> emit them now so they run during the edges GCN
                    for w in range(NT // FINW, C // FINW):
                        w0 = w * FINW
                        tch = tp.tile([P, FINW, H], bft)
                        nc.sync.dma_start(
                            out=tch[:], in_=ins_d["t_seg"][:, w0 : w0 + FINW, :]
                        )
                        och = finp.tile([P, FINW, H], f32)
                        nc.vector.tensor_tensor(
                            out=och[:], in0=tch[:], in1=en_all[:, w0 : w0 + FINW, :],
                            op=mybir.AluOpType.add,
                        )
                        nc.sync.dma_start(
                            out=out_d[:, w0 : w0 + FINW, :], in_=och[:]
                        )


            # ---------- final combine for the edges-image rows ----------
            for w in range(NT // FINW):
                w0 = w * FINW
                tch = tp.tile([P, FINW, H], bft)
                nc.sync.dma_start(out=tch[:], in_=ins_d["t_seg"][:, w0 : w0 + FINW, :])
                och = finp.tile([P, FINW, H], f32)
                nc.vector.tensor_tensor(
                    out=och[:], in0=tch[:], in1=en_all[:, w0 : w0 + FINW, :],
                    op=mybir.AluOpType.add,
                )
                nc.vector.tensor_tensor(
                    out=och[:], in0=och[:], in1=e_sb["e"][w][:],
                    op=mybir.AluOpType.add,
                )
                nc.sync.dma_start(out=out_d[:, w0 : w0 + FINW, :], in_=och[:])

    nc.compile()

    in_maps = [{k: v for k, v in cd.items()} for cd in core_data]
    global LAST_RESULT
    kw = {}
    if TRACE:
        kw = dict(trace=True, trace_cores=TRACE_CORES, stitch_traces=False)
    res = run_bass_kernel_spmd(nc, in_maps, list(range(8)), **kw)
    LAST_RESULT = res

    out = np.empty((B, S, D), np.float32)
    for b in range(B):
        sd = samp[b]
        n2 = sd["len2"]
        for h in range(2):
            o = np.asarray(res.results[2 * b + h]["out"], dtype=np.float32)
            rows = o.transpose(1, 0, 2).reshape(C * P, H)
            out[b, x2t["e"][b], h * H : (h + 1) * H] = rows[:N]
            out[b, sd["seg2_tok"], h * H : (h + 1) * H] = rows[N : N + n2]
        out[b, sd["seg3_tok"], :] = t_full[b, sd["seg3_tok"], :]
    return out



# revision 3
# speedup vs baseline: 1.4477x; 1.4477x over previous
"""Trainium2 Bass kernel for CausalMessagePassingLayer (2x GCN + gated scatter).

Sharding: 8 cores = 4 samples x 2 halves of the embedding dim (D=768 -> 384).
Each core is fully independent (no collectives).

v3 pipeline (graph renumbering kills all gathers / DRAM bounces):
  - host renumbers each subgraph's nodes so that tokens present in BOTH
    images sit at the SAME row position in the edges-GCN and nodes-GCN
    output spaces (class-i rows at positions 0..|i|-1 of both).  The final
    combine then reads both GCN outputs as contiguous SBUF tiles - no
    dma_gather, no DRAM roundtrip for e_nodes, no serial valley.
  - xw: y = dinv_src * (x @ W[:, half]) via bf16 matmuls (xT is chunk-
    loaded so the first matmul starts ~2us in), y kept in SBUF as fp8
  - GCN aggregation z[dst] += count * y[src] as dense per-tile adjacency
    blocks (fp8 counts, incl. self-loops) with DoubleRow pairs of src
    tiles accumulating in PSUM; epilogue -> e rows bf16 in SBUF
  - output rows: region A = all 4096 edges-image rows (out = t + e_e +
    mask*e_n, mask zeroes e_n for rows whose token is not in the nodes
    image); region B = nodes-row chunks >= LI0 (out = t + e_n), covering
    all nodes-only tokens (duplicated class-i rows are dropped on host)
  - region-B waves are emitted inside the nodes agg loop, region-A waves
    inside the edges agg loop, so out-writes stream during aggregation
  - tokens in neither image are exact f32 t passthrough assembled on host

Host-side work is restricted to index/descriptor preparation (renumbering,
dense count blocks, degree counts, masks, inverse permutations) and
dtype/layout marshalling of inputs; all floating-point math runs on device.
"""

import numpy as np
import ml_dtypes

B, S, D, N, E = 4, 8192, 768, 4096, 32768
H = D // 2            # per-core half of embedding dim
P = 128
NT = N // P           # 32 node tiles per graph
KT = D // P           # 6 k-tiles of the contraction dim
FINW = 4              # chunks per final wave

bf16 = ml_dtypes.bfloat16
f8 = ml_dtypes.float8_e4m3

# test-harness knobs (the grading harness just calls kernel() and these stay default)
TRACE = False
TRACE_CORES = None
LAST_RESULT = None


def _prep_graph(ei):
    """Dense adjacency-count blocks (incl. self loops) + degree counts.

    Returns (A_blocks, deg): A_blocks[t, p, sc, q] = #edges src=sc*128+p ->
    dst=t*128+q, laid out so A_blocks[t] is directly the stack of matmul
    lhsT tiles for dst-tile t. deg includes the self loop.
    """
    s = np.concatenate([ei[0].astype(np.int64), np.arange(N, dtype=np.int64)])
    d = np.concatenate([ei[1].astype(np.int64), np.arange(N, dtype=np.int64)])
    A = np.zeros((N, N), np.float32)
    np.add.at(A, (d, s), 1.0)
    deg = np.bincount(d, minlength=N).astype(np.int32)
    Ab = np.ascontiguousarray(
        A.reshape(NT, P, NT, P).transpose(0, 3, 2, 1)
    ).astype(f8)
    return Ab, deg


def _xT(x):
    """[N, D] -> bf16 [P, KT, N] with K element d = s*128 + p."""
    return np.ascontiguousarray(x.T.reshape(KT, P, N).transpose(1, 0, 2)).astype(bf16)


def kernel(**inputs):
    import concourse.bacc as bacc
    import concourse.mybir as mybir
    import concourse.tile as tile
    from concourse.bass_utils import run_bass_kernel_spmd

    f32, bft, fp8, i32 = (
        mybir.dt.float32,
        mybir.dt.bfloat16,
        mybir.dt.float8e4,
        mybir.dt.int32,
    )
    DR = mybir.MatmulPerfMode.DoubleRow

    t_full = np.asarray(inputs["token_embeddings"], dtype=np.float32)
    W = {
        "e": np.asarray(inputs["W_edges"], dtype=np.float32),
        "n": np.asarray(inputs["W_nodes"], dtype=np.float32),
    }
    bias = {
        "e": np.asarray(inputs["b_edges"], dtype=np.float32),
        "n": np.asarray(inputs["b_nodes"], dtype=np.float32),
    }
    gate = {
        "e": np.asarray(inputs["gate_a"], dtype=np.float32).reshape(1, 1),
        "n": np.asarray(inputs["gate_b"], dtype=np.float32).reshape(1, 1),
    }
    t2x = {
        "e": np.asarray(inputs["tokens2edges"], dtype=np.int64),
        "n": np.asarray(inputs["tokens2nodes"], dtype=np.int64),
    }
    x2t = {
        "e": np.asarray(inputs["edges2tokens"], dtype=np.int64),
        "n": np.asarray(inputs["nodes2tokens"], dtype=np.int64),
    }
    ei = {
        "e": np.asarray(inputs["edge_index_edges"], dtype=np.int64),
        "n": np.asarray(inputs["edge_index_nodes"], dtype=np.int64),
    }

    gcns = ("n", "e")  # nodes first: region-B waves stream during agg_n

    # ---- per-sample host index prep (graph renumbering) ----
    samp = []
    for b in range(B):
        sd = {}
        e_img = x2t["e"][b]                      # token of old e-row i
        n_img = x2t["n"][b]
        r_e = np.full(S, -1, np.int64); r_e[e_img] = np.arange(N)
        r_n = np.full(S, -1, np.int64); r_n[n_img] = np.arange(N)
        in_e = r_e >= 0
        in_n = r_n >= 0
        toks_i = np.where(in_e & in_n)[0]        # class i (both images), sorted
        ni = len(toks_i)
        sd["ni"] = ni
        # sigma_e: old e-row -> new position; class i -> 0..ni-1 (token order)
        sig_e = np.full(N, -1, np.int64)
        sig_e[r_e[toks_i]] = np.arange(ni)
        rest_e = np.where(sig_e < 0)[0]          # class ii old rows
        sig_e[rest_e] = ni + np.arange(N - ni)
        # sigma_n: old n-row -> new position; class i -> 0..ni-1 (same order)
        sig_n = np.full(N, -1, np.int64)
        sig_n[r_n[toks_i]] = np.arange(ni)
        rest_n = np.where(sig_n < 0)[0]          # class iii old rows
        sig_n[rest_n] = ni + np.arange(N - ni)
        sd["sig"] = {"e": sig_e, "n": sig_n}
        # token of each NEW position
        inv_e = np.empty(N, np.int64); inv_e[sig_e] = np.arange(N)
        inv_n = np.empty(N, np.int64); inv_n[sig_n] = np.arange(N)
        sd["tokA"] = e_img[inv_e]                # region-A slot j -> token
        sd["tokB"] = n_img[inv_n]                # n-position p -> token
        for g in ("e", "n"):
            eig = sd["sig"][g][ei[g][b]]         # renumbered edge index
            sd[f"A_{g}"], sd[f"deg_{g}"] = _prep_graph(eig)
        sd["seg3_tok"] = np.where(~in_e & ~in_n)[0]
        samp.append(sd)

    LI0 = min(sd["ni"] // P for sd in samp)      # common region-B start chunk
    C = NT + (NT - LI0)                          # output chunks per core

    for b, sd in enumerate(samp):
        ni = sd["ni"]
        # mask[p, c] = 1 iff e_n row (c*128+p) is class i (gets added in region A)
        m = (np.arange(N) < ni).astype(np.float32).reshape(NT, P).T
        sd["mask"] = np.ascontiguousarray(m)
        # t rows in final slot order [4096 region A | (NT-LI0)*128 region B]
        rows = np.zeros((C * P, D), np.float32)
        rows[:N] = t_full[b][sd["tokA"]]
        rows[N:] = t_full[b][sd["tokB"][LI0 * P :]]
        sd["t_rows"] = rows

    # ---- per-core host data ----
    core_data = []
    for b in range(B):
        sd = samp[b]
        for h in range(2):
            d = {}
            for g in gcns:
                xg = t_full[b][t2x[g][b]]        # old x rows
                inv = np.empty(N, np.int64); inv[sd["sig"][g]] = np.arange(N)
                d[f"xT_{g}"] = _xT(xg[inv])      # renumbered row order
                d[f"W_{g}"] = np.ascontiguousarray(
                    W[g][:, h * H : (h + 1) * H].reshape(KT, P, H).transpose(1, 0, 2)
                ).astype(bf16)
                d[f"bias_{g}"] = np.ascontiguousarray(bias[g][None, h * H : (h + 1) * H])
                d[f"gate_{g}"] = gate[g]
                d[f"A_{g}"] = sd[f"A_{g}"]
                d[f"deg_pc_{g}"] = np.ascontiguousarray(
                    sd[f"deg_{g}"].reshape(NT, P).T
                )
            d["mask"] = sd["mask"]
            d["t_seg"] = np.ascontiguousarray(
                sd["t_rows"].reshape(C, P, D).transpose(1, 0, 2)[:, :, h * H : (h + 1) * H]
            ).astype(bf16)
            core_data.append(d)

    # ---- build the SPMD program ----
    nc = bacc.Bacc("TRN2", target_bir_lowering=False, debug=False, num_swdge_queues=1)

    ins_d = {}
    for g in gcns:
        ins_d[f"xT_{g}"] = nc.declare_dram_parameter(f"xT_{g}", [P, KT, N], bft, isOutput=False)
        ins_d[f"W_{g}"] = nc.declare_dram_parameter(f"W_{g}", [P, KT, H], bft, isOutput=False)
        ins_d[f"bias_{g}"] = nc.declare_dram_parameter(f"bias_{g}", [1, H], f32, isOutput=False)
        ins_d[f"gate_{g}"] = nc.declare_dram_parameter(f"gate_{g}", [1, 1], f32, isOutput=False)
        ins_d[f"A_{g}"] = nc.declare_dram_parameter(
            f"A_{g}", [NT, P, NT, P], fp8, isOutput=False
        )
        ins_d[f"deg_pc_{g}"] = nc.declare_dram_parameter(
            f"deg_pc_{g}", [P, NT], i32, isOutput=False
        )
    ins_d["mask"] = nc.declare_dram_parameter("mask", [P, NT], f32, isOutput=False)
    ins_d["t_seg"] = nc.declare_dram_parameter("t_seg", [P, C, H], bft, isOutput=False)
    out_d = nc.declare_dram_parameter("out", [P, C, H], f32, isOutput=True)

    NW = NT // FINW       # 8 e_sb wave-tiles per GCN

    with tile.TileContext(nc) as tc:
        with (
            tc.tile_pool(name="cst", bufs=1) as cst,
            tc.tile_pool(name="xt", bufs=1) as xtp,
            tc.tile_pool(name="yp", bufs=1) as yp,
            tc.tile_pool(name="ap", bufs=3) as apool,
            tc.tile_pool(name="epn", bufs=1) as epn,
            tc.tile_pool(name="epe", bufs=1) as epe,
            tc.tile_pool(name="tpa", bufs=3) as tpa,
            tc.tile_pool(name="tpb", bufs=2) as tpb,
            tc.tile_pool(name="foa", bufs=2) as foa,
            tc.tile_pool(name="fob", bufs=2) as fob,
            tc.tile_pool(name="psxw", bufs=2, space="PSUM") as psxw,
            tc.tile_pool(name="psz", bufs=3, space="PSUM") as psz,
        ):
            # ---------- setup ----------
            Wsb, bias_ga, dinv, dinv_ga = {}, {}, {}, {}
            for g in gcns:
                Wsb[g] = cst.tile([P, KT, H], bft, name=f"W_{g}", tag=f"W_{g}")
                nc.sync.dma_start(out=Wsb[g][:], in_=ins_d[f"W_{g}"][:])

                gcol = cst.tile([P, 1], f32, name=f"gcol_{g}", tag=f"gcol_{g}")
                nc.sync.dma_start(
                    out=gcol[:], in_=ins_d[f"gate_{g}"][:1, :].to_broadcast([P, 1])
                )
                tanh_g = cst.tile([P, 1], f32, name=f"tanh_{g}", tag=f"tanh_{g}")
                nc.scalar.activation(
                    out=tanh_g[:], in_=gcol[:], func=mybir.ActivationFunctionType.Tanh
                )

                brow = cst.tile([P, H], f32, name=f"brow_{g}", tag=f"brow_{g}")
                nc.sync.dma_start(
                    out=brow[:], in_=ins_d[f"bias_{g}"][:1, :].to_broadcast([P, H])
                )
                bias_ga[g] = cst.tile([P, H], f32, name=f"biasga_{g}", tag=f"biasga_{g}")
                nc.vector.tensor_scalar_mul(bias_ga[g][:], brow[:], tanh_g[:, :1])

                deg_i = cst.tile([P, NT], i32, name=f"degi_{g}", tag=f"degi_{g}")
                nc.sync.dma_start(out=deg_i[:], in_=ins_d[f"deg_pc_{g}"][:])
                deg_f = cst.tile([P, NT], f32, name=f"degf_{g}", tag=f"degf_{g}")
                nc.vector.tensor_copy(out=deg_f[:], in_=deg_i[:])
                rdeg = cst.tile([P, NT], f32, name=f"rdeg_{g}", tag=f"rdeg_{g}")
                nc.vector.reciprocal(rdeg[:], deg_f[:])
                dinv[g] = cst.tile([P, NT], f32, name=f"dinv_{g}", tag=f"dinv_{g}")
                nc.scalar.sqrt(dinv[g][:], rdeg[:])
                dinv_ga[g] = cst.tile([P, NT], f32, name=f"dinvga_{g}", tag=f"dinvga_{g}")
                nc.vector.tensor_scalar_mul(dinv_ga[g][:], dinv[g][:], tanh_g[:, :1])

            mask = cst.tile([P, NT], f32, name="mask", tag="mask")
            nc.sync.dma_start(out=mask[:], in_=ins_d["mask"][:])

            e_sb = {
                g: [
                    (epn if g == "n" else epe).tile(
                        [P, FINW, H], bft, name=f"esb_{g}_{w}", tag=f"esb_{g}_{w}"
                    )
                    for w in range(NW)
                ]
                for g in gcns
            }

            # ---------- per-GCN ----------
            for g in gcns:
                xT = xtp.tile([P, KT, N], bft, tag="xT")
                NXC = 4
                for xc in range(NXC):
                    nc.sync.dma_start(
                        out=xT[:, :, xc * (N // NXC) : (xc + 1) * (N // NXC)],
                        in_=ins_d[f"xT_{g}"][:, :, xc * (N // NXC) : (xc + 1) * (N // NXC)],
                    )

                y_sb = yp.tile([P, NT, H], fp8, tag="ysb")
                for c in range(NT):
                    ps = psxw.tile([P, H], f32)
                    for j in range(KT):
                        nc.tensor.matmul(
                            out=ps[:],
                            lhsT=xT[:, j, c * P : (c + 1) * P],
                            rhs=Wsb[g][:, j, :],
                            start=(j == 0),
                            stop=(j == KT - 1),
                        )
                    nc.scalar.activation(
                        out=y_sb[:, c, :],
                        in_=ps[:],
                        func=mybir.ActivationFunctionType.Copy,
                        scale=dinv[g][:, c : c + 1],
                    )

                # aggregation via dense per-tile adjacency count blocks
                for t_i in range(NT):
                    At = apool.tile([P, NT, P], fp8, name="At", tag="At")
                    nc.sync.dma_start(out=At[:], in_=ins_d[f"A_{g}"][t_i])
                    zt = psz.tile([P, H], f32, name="zt", tag="zt")
                    for j in range(NT // 2):
                        nc.tensor.matmul(
                            out=zt[:],
                            lhsT=At[:, 2 * j : 2 * j + 2, :],
                            rhs=y_sb[:, 2 * j : 2 * j + 2, :],
                            start=(j == 0),
                            stop=(j == NT // 2 - 1),
                            perf_mode=DR,
                        )
                    w, c = t_i // FINW, t_i % FINW
                    nc.vector.scalar_tensor_tensor(
                        out=e_sb[g][w][:, c, :],
                        in0=zt[:],
                        scalar=dinv_ga[g][:, t_i : t_i + 1],
                        in1=bias_ga[g][:],
                        op0=mybir.AluOpType.mult,
                        op1=mybir.AluOpType.add,
                    )
                    if c == FINW - 1:
                        if g == "n" and t_i >= LI0:
                            # region-B wave: out = t + e_n for position chunks
                            # [c0, c1) - nodes-only tokens (+ ignored dups)
                            c0, c1 = max(LI0, w * FINW), (w + 1) * FINW
                            o0 = NT + c0 - LI0
                            nw = c1 - c0
                            tch = tpb.tile([P, FINW, H], bft, name="tchb", tag="tchb")
                            nc.sync.dma_start(
                                out=tch[:, :nw, :],
                                in_=ins_d["t_seg"][:, o0 : o0 + nw, :],
                            )
                            och = fob.tile([P, FINW, H], f32, name="ochb", tag="ochb")
                            nc.vector.tensor_tensor(
                                out=och[:, :nw, :],
                                in0=tch[:, :nw, :],
                                in1=e_sb["n"][w][:, FINW - nw :, :],
                                op=mybir.AluOpType.add,
                            )
                            nc.sync.dma_start(
                                out=out_d[:, o0 : o0 + nw, :], in_=och[:, :nw, :]
                            )
                        if g == "e":
                            # region-A wave: out = t + e_e + mask*e_n
                            w0 = w * FINW
                            tch = tpa.tile([P, FINW, H], bft, name="tcha", tag="tcha")
                            nc.sync.dma_start(
                                out=tch[:], in_=ins_d["t_seg"][:, w0 : w0 + FINW, :]
                            )
                            och = foa.tile([P, FINW, H], f32, name="ocha", tag="ocha")
                            nc.vector.tensor_tensor(
                                out=och[:], in0=tch[:], in1=e_sb["e"][w][:],
                                op=mybir.AluOpType.add,
                            )
                            for cc in range(FINW):
                                nc.vector.scalar_tensor_tensor(
                                    out=och[:, cc, :],
                                    in0=e_sb["n"][w][:, cc, :],
                                    scalar=mask[:, w0 + cc : w0 + cc + 1],
                                    in1=och[:, cc, :],
                                    op0=mybir.AluOpType.mult,
                                    op1=mybir.AluOpType.add,
                                )
                            nc.sync.dma_start(
                                out=out_d[:, w0 : w0 + FINW, :], in_=och[:]
                            )

    nc.compile()

    in_maps = [{k: v for k, v in cd.items()} for cd in core_data]
    global LAST_RESULT
    kw = {}
    if TRACE:
        kw = dict(trace=True, trace_cores=TRACE_CORES, stitch_traces=False)
    res = run_bass_kernel_spmd(nc, in_maps, list(range(8)), **kw)
    LAST_RESULT = res

    out = np.empty((B, S, D), np.float32)
    for b in range(B):
        sd = samp[b]
        ni = sd["ni"]
        tokB_valid = sd["tokB"][ni:]             # class-iii tokens
        boff = N + (ni - LI0 * P)                # their first output row
        for h in range(2):
            o = np.asarray(res.results[2 * b + h]["out"], dtype=np.float32)
            rows = o.transpose(1, 0, 2).reshape(C * P, H)
            out[b, sd["tokA"], h * H : (h + 1) * H] = rows[:N]
            out[b, tokB_valid, h * H : (h + 1) * H] = rows[boff : boff + len(tokB_valid)]
        out[b, sd["seg3_tok"], :] = t_full[b, sd["seg3_tok"], :]
    return out


# revision 10
# speedup vs baseline: 1.6527x; 1.1416x over previous
"""Trainium2 Bass kernel for CausalMessagePassingLayer (2x GCN + gated scatter).

Sharding: 8 cores = 4 samples x 2 halves of the embedding dim (D=768 -> 384).
Each core is fully independent (no collectives).

v3 pipeline (graph renumbering kills all gathers / DRAM bounces):
  - host renumbers each subgraph's nodes so that tokens present in BOTH
    images sit at the SAME row position in the edges-GCN and nodes-GCN
    output spaces (class-i rows at positions 0..|i|-1 of both).  The final
    combine then reads both GCN outputs as contiguous SBUF tiles - no
    dma_gather, no DRAM roundtrip for e_nodes, no serial valley.
  - xw: y = dinv_src * (x @ W[:, half]) via bf16 matmuls (xT is chunk-
    loaded so the first matmul starts ~2us in), y kept in SBUF as fp8
  - GCN aggregation z[dst] += count * y[src] as dense per-tile adjacency
    blocks (fp8 counts, incl. self-loops) with DoubleRow pairs of src
    tiles accumulating in PSUM; epilogue -> e rows bf16 in SBUF
  - output rows: region A = all 4096 edges-image rows (out = t + e_e +
    mask*e_n, mask zeroes e_n for rows whose token is not in the nodes
    image); region B = nodes-row chunks >= LI0 (out = t + e_n), covering
    all nodes-only tokens (duplicated class-i rows are dropped on host)
  - region-B waves and region-A pre-sums (p = t + mask*e_n) run in the
    xw_e window; each region-A wave inside the edges agg loop is then a
    single add p + e_e, so agg windows stream only A blocks + out rows
  - out rows are written bf16 (host converts back to f32); tokens in
    neither image are exact f32 t passthrough assembled on host

Host-side work is restricted to index/descriptor preparation (renumbering,
dense count blocks, degree counts, masks, inverse permutations) and
dtype/layout marshalling of inputs; all floating-point math runs on device.
"""

import numpy as np
import ml_dtypes

B, S, D, N, E = 4, 8192, 768, 4096, 32768
H = D // 2            # per-core half of embedding dim
P = 128
NT = N // P           # 32 node tiles per graph
KT = D // P           # 6 k-tiles of the contraction dim
FINW = 4              # chunks per final wave

NXC = 8               # xT load pieces (512 columns each)

bf16 = ml_dtypes.bfloat16
f8 = ml_dtypes.float8_e4m3

# test-harness knobs (the grading harness just calls kernel() and these stay default)
TRACE = False
TRACE_CORES = None
LAST_RESULT = None


def _prep_graph(ei):
    """Dense adjacency-count blocks (incl. self loops) + degree counts.

    Returns (A_blocks, deg): A_blocks[t, p, sc, q] = #edges src=sc*128+p ->
    dst=t*128+q, laid out so A_blocks[t] is directly the stack of matmul
    lhsT tiles for dst-tile t. deg includes the self loop.
    """
    s = np.concatenate([ei[0].astype(np.int64), np.arange(N, dtype=np.int64)])
    d = np.concatenate([ei[1].astype(np.int64), np.arange(N, dtype=np.int64)])
    A = np.zeros((N, N), np.float32)
    np.add.at(A, (d, s), 1.0)
    deg = np.bincount(d, minlength=N).astype(np.int32)
    Ab = np.ascontiguousarray(
        A.reshape(NT, P, NT, P).transpose(0, 3, 2, 1)
    ).astype(f8)
    return Ab, deg


def _xT(x):
    """[N, D] -> bf16 [P, KT, N] with K element d = s*128 + p."""
    return np.ascontiguousarray(x.T.reshape(KT, P, N).transpose(1, 0, 2)).astype(bf16)


def kernel(**inputs):
    import concourse.bacc as bacc
    import concourse.mybir as mybir
    import concourse.tile as tile
    from concourse.bass_utils import run_bass_kernel_spmd

    f32, bft, fp8, i32 = (
        mybir.dt.float32,
        mybir.dt.bfloat16,
        mybir.dt.float8e4,
        mybir.dt.int32,
    )
    DR = mybir.MatmulPerfMode.DoubleRow

    t_full = np.asarray(inputs["token_embeddings"], dtype=np.float32)
    W = {
        "e": np.asarray(inputs["W_edges"], dtype=np.float32),
        "n": np.asarray(inputs["W_nodes"], dtype=np.float32),
    }
    bias = {
        "e": np.asarray(inputs["b_edges"], dtype=np.float32),
        "n": np.asarray(inputs["b_nodes"], dtype=np.float32),
    }
    gate = {
        "e": np.asarray(inputs["gate_a"], dtype=np.float32).reshape(1, 1),
        "n": np.asarray(inputs["gate_b"], dtype=np.float32).reshape(1, 1),
    }
    t2x = {
        "e": np.asarray(inputs["tokens2edges"], dtype=np.int64),
        "n": np.asarray(inputs["tokens2nodes"], dtype=np.int64),
    }
    x2t = {
        "e": np.asarray(inputs["edges2tokens"], dtype=np.int64),
        "n": np.asarray(inputs["nodes2tokens"], dtype=np.int64),
    }
    ei = {
        "e": np.asarray(inputs["edge_index_edges"], dtype=np.int64),
        "n": np.asarray(inputs["edge_index_nodes"], dtype=np.int64),
    }

    gcns = ("n", "e")  # nodes first: region-B waves stream during agg_n

    # ---- per-sample host index prep (graph renumbering) ----
    samp = []
    for b in range(B):
        sd = {}
        e_img = x2t["e"][b]                      # token of old e-row i
        n_img = x2t["n"][b]
        r_e = np.full(S, -1, np.int64); r_e[e_img] = np.arange(N)
        r_n = np.full(S, -1, np.int64); r_n[n_img] = np.arange(N)
        in_e = r_e >= 0
        in_n = r_n >= 0
        toks_i = np.where(in_e & in_n)[0]        # class i (both images), sorted
        ni = len(toks_i)
        sd["ni"] = ni
        # sigma_e: old e-row -> new position; class i -> 0..ni-1 (token order)
        sig_e = np.full(N, -1, np.int64)
        sig_e[r_e[toks_i]] = np.arange(ni)
        rest_e = np.where(sig_e < 0)[0]          # class ii old rows
        sig_e[rest_e] = ni + np.arange(N - ni)
        # sigma_n: old n-row -> new position; class i -> 0..ni-1 (same order)
        sig_n = np.full(N, -1, np.int64)
        sig_n[r_n[toks_i]] = np.arange(ni)
        rest_n = np.where(sig_n < 0)[0]          # class iii old rows
        sig_n[rest_n] = ni + np.arange(N - ni)
        sd["sig"] = {"e": sig_e, "n": sig_n}
        # token of each NEW position
        inv_e = np.empty(N, np.int64); inv_e[sig_e] = np.arange(N)
        inv_n = np.empty(N, np.int64); inv_n[sig_n] = np.arange(N)
        sd["tokA"] = e_img[inv_e]                # region-A slot j -> token
        sd["tokB"] = n_img[inv_n]                # n-position p -> token
        for g in ("e", "n"):
            eig = sd["sig"][g][ei[g][b]]         # renumbered edge index
            sd[f"A_{g}"], sd[f"deg_{g}"] = _prep_graph(eig)
        sd["seg3_tok"] = np.where(~in_e & ~in_n)[0]
        samp.append(sd)

    LI0 = min(sd["ni"] // P for sd in samp)      # common region-B start chunk
    C = NT + (NT - LI0)                          # output chunks per core

    for b, sd in enumerate(samp):
        ni = sd["ni"]
        # mask[p, c] = 1 iff e_n row (c*128+p) is class i (gets added in region A)
        m = (np.arange(N) < ni).astype(np.float32).reshape(NT, P).T
        sd["mask"] = np.ascontiguousarray(m)
        # t rows in final slot order [4096 region A | (NT-LI0)*128 region B]
        rows = np.zeros((C * P, D), np.float32)
        rows[:N] = t_full[b][sd["tokA"]]
        rows[N:] = t_full[b][sd["tokB"][LI0 * P :]]
        sd["t_rows"] = rows

    # ---- per-core host data ----
    core_data = []
    for b in range(B):
        sd = samp[b]
        for h in range(2):
            d = {}
            for g in gcns:
                xg = t_full[b][t2x[g][b]]        # old x rows
                inv = np.empty(N, np.int64); inv[sd["sig"][g]] = np.arange(N)
                d[f"xT_{g}"] = _xT(xg[inv])      # renumbered row order
                d[f"W_{g}"] = np.ascontiguousarray(
                    W[g][:, h * H : (h + 1) * H].reshape(KT, P, H).transpose(1, 0, 2)
                ).astype(bf16)
                d[f"bias_{g}"] = np.ascontiguousarray(bias[g][None, h * H : (h + 1) * H])
                d[f"gate_{g}"] = gate[g]
                d[f"A_{g}"] = sd[f"A_{g}"]
                d[f"deg_pc_{g}"] = np.ascontiguousarray(
                    sd[f"deg_{g}"].reshape(NT, P).T
                )
            d["mask"] = sd["mask"]
            d["t_seg"] = np.ascontiguousarray(
                sd["t_rows"].reshape(C, P, D).transpose(1, 0, 2)[:, :, h * H : (h + 1) * H]
            ).astype(bf16)
            for g in gcns:
                # piece-contiguous layout: [P, NXC, KT, N/NXC] so each DMA
                # piece is one contiguous run per partition
                d[f"xT_{g}"] = np.ascontiguousarray(
                    d[f"xT_{g}"].reshape(P, KT, NXC, N // NXC).transpose(0, 2, 1, 3)
                )
            core_data.append(d)

    # ---- build the SPMD program ----
    nc = bacc.Bacc("TRN2", target_bir_lowering=False, debug=False, num_swdge_queues=1)

    ins_d = {}
    for g in gcns:
        ins_d[f"xT_{g}"] = nc.declare_dram_parameter(
            f"xT_{g}", [P, NXC, KT, N // NXC], bft, isOutput=False
        )
        ins_d[f"W_{g}"] = nc.declare_dram_parameter(f"W_{g}", [P, KT, H], bft, isOutput=False)
        ins_d[f"bias_{g}"] = nc.declare_dram_parameter(f"bias_{g}", [1, H], f32, isOutput=False)
        ins_d[f"gate_{g}"] = nc.declare_dram_parameter(f"gate_{g}", [1, 1], f32, isOutput=False)
        ins_d[f"A_{g}"] = nc.declare_dram_parameter(
            f"A_{g}", [NT, P, NT, P], fp8, isOutput=False
        )
        ins_d[f"deg_pc_{g}"] = nc.declare_dram_parameter(
            f"deg_pc_{g}", [P, NT], i32, isOutput=False
        )
    ins_d["mask"] = nc.declare_dram_parameter("mask", [P, NT], f32, isOutput=False)
    ins_d["t_seg"] = nc.declare_dram_parameter("t_seg", [P, C, H], bft, isOutput=False)
    out_d = nc.declare_dram_parameter("out", [P, C, H], bft, isOutput=True)

    NW = NT // FINW       # 8 e_sb wave-tiles per GCN

    with tile.TileContext(nc) as tc:
        with (
            tc.tile_pool(name="cst", bufs=1) as cst,
            tc.tile_pool(name="xt", bufs=1) as xtp,
            tc.tile_pool(name="yp", bufs=1) as yp,
            tc.tile_pool(name="ap", bufs=4) as apool,
            tc.tile_pool(name="epn", bufs=1) as epn,
            tc.tile_pool(name="epe", bufs=1) as epe,
            tc.tile_pool(name="pwp", bufs=1) as pwp,
            tc.tile_pool(name="tpa", bufs=3) as tpa,
            tc.tile_pool(name="tpb", bufs=2) as tpb,
            tc.tile_pool(name="foa", bufs=2) as foa,
            tc.tile_pool(name="fob", bufs=2) as fob,
            tc.tile_pool(name="psxw", bufs=2, space="PSUM") as psxw,
            tc.tile_pool(name="psz", bufs=3, space="PSUM") as psz,
        ):
            # ---------- setup ----------
            Wsb, bias_ga, dinv, dinv_ga = {}, {}, {}, {}
            for g in gcns:
                Wsb[g] = cst.tile([P, KT, H], bft, name=f"W_{g}", tag=f"W_{g}")
                nc.sync.dma_start(out=Wsb[g][:], in_=ins_d[f"W_{g}"][:])

                gcol = cst.tile([P, 1], f32, name=f"gcol_{g}", tag=f"gcol_{g}")
                nc.sync.dma_start(
                    out=gcol[:], in_=ins_d[f"gate_{g}"][:1, :].to_broadcast([P, 1])
                )
                tanh_g = cst.tile([P, 1], f32, name=f"tanh_{g}", tag=f"tanh_{g}")
                nc.scalar.activation(
                    out=tanh_g[:], in_=gcol[:], func=mybir.ActivationFunctionType.Tanh
                )

                brow = cst.tile([P, H], f32, name=f"brow_{g}", tag=f"brow_{g}")
                nc.sync.dma_start(
                    out=brow[:], in_=ins_d[f"bias_{g}"][:1, :].to_broadcast([P, H])
                )
                bias_ga[g] = cst.tile([P, H], f32, name=f"biasga_{g}", tag=f"biasga_{g}")
                nc.vector.tensor_scalar_mul(bias_ga[g][:], brow[:], tanh_g[:, :1])

                deg_i = cst.tile([P, NT], i32, name=f"degi_{g}", tag=f"degi_{g}")
                nc.sync.dma_start(out=deg_i[:], in_=ins_d[f"deg_pc_{g}"][:])
                deg_f = cst.tile([P, NT], f32, name=f"degf_{g}", tag=f"degf_{g}")
                nc.vector.tensor_copy(out=deg_f[:], in_=deg_i[:])
                rdeg = cst.tile([P, NT], f32, name=f"rdeg_{g}", tag=f"rdeg_{g}")
                nc.vector.reciprocal(rdeg[:], deg_f[:])
                dinv[g] = cst.tile([P, NT], f32, name=f"dinv_{g}", tag=f"dinv_{g}")
                nc.scalar.sqrt(dinv[g][:], rdeg[:])
                dinv_ga[g] = cst.tile([P, NT], f32, name=f"dinvga_{g}", tag=f"dinvga_{g}")
                nc.vector.tensor_scalar_mul(dinv_ga[g][:], dinv[g][:], tanh_g[:, :1])

            mask = cst.tile([P, NT], f32, name="mask", tag="mask")
            nc.sync.dma_start(out=mask[:], in_=ins_d["mask"][:])

            e_sb = {
                g: [
                    (epn if g == "n" else epe).tile(
                        [P, FINW, H], bft, name=f"esb_{g}_{w}", tag=f"esb_{g}_{w}"
                    )
                    for w in range(NW)
                ]
                for g in gcns
            }

            # ---------- per-GCN ----------
            pw = []
            for g in gcns:
                xT = xtp.tile([P, NXC, KT, N // NXC], bft, tag="xT")
                for xc in range(NXC):
                    nc.sync.dma_start(out=xT[:, xc], in_=ins_d[f"xT_{g}"][:, xc])

                y_sb = yp.tile([P, NT, H], fp8, tag="ysb")
                CPX = NT // NXC       # xw chunks per xT piece
                for c in range(NT):
                    pc, lc = c // CPX, c % CPX
                    ps = psxw.tile([P, H], f32)
                    for j in range(KT):
                        nc.tensor.matmul(
                            out=ps[:],
                            lhsT=xT[:, pc, j, lc * P : (lc + 1) * P],
                            rhs=Wsb[g][:, j, :],
                            start=(j == 0),
                            stop=(j == KT - 1),
                        )
                    nc.scalar.activation(
                        out=y_sb[:, c, :],
                        in_=ps[:],
                        func=mybir.ActivationFunctionType.Copy,
                        scale=dinv[g][:, c : c + 1],
                    )

                if g == "e":
                    # issued after xw_e, so this vector+DMA work overlaps the
                    # xw_e PE stream (e_sb_n is complete by now)
                    for w in range(LI0 // FINW, NW):
                        # region-B wave: out = t + e_n for position chunks
                        # [c0, c1) - nodes-only tokens (+ ignored dups)
                        c0, c1 = max(LI0, w * FINW), (w + 1) * FINW
                        o0 = NT + c0 - LI0
                        nw = c1 - c0
                        tch = tpb.tile([P, FINW, H], bft, name="tchb", tag="tchb")
                        nc.sync.dma_start(
                            out=tch[:, :nw, :],
                            in_=ins_d["t_seg"][:, o0 : o0 + nw, :],
                        )
                        och = fob.tile([P, FINW, H], bft, name="ochb", tag="ochb")
                        nc.vector.tensor_tensor(
                            out=och[:, :nw, :],
                            in0=tch[:, :nw, :],
                            in1=e_sb["n"][w][:, FINW - nw :, :],
                            op=mybir.AluOpType.add,
                        )
                        nc.sync.dma_start(
                            out=out_d[:, o0 : o0 + nw, :], in_=och[:, :nw, :]
                        )
                    for w in range(NW):
                        # region-A pre-sum p_w = t + mask*e_n
                        w0 = w * FINW
                        tch = tpa.tile([P, FINW, H], bft, name="tcha", tag="tcha")
                        nc.sync.dma_start(
                            out=tch[:], in_=ins_d["t_seg"][:, w0 : w0 + FINW, :]
                        )
                        p = pwp.tile([P, FINW, H], bft, name=f"pw{w}", tag=f"pw{w}")
                        for cc in range(FINW):
                            nc.vector.scalar_tensor_tensor(
                                out=p[:, cc, :],
                                in0=e_sb["n"][w][:, cc, :],
                                scalar=mask[:, w0 + cc : w0 + cc + 1],
                                in1=tch[:, cc, :],
                                op0=mybir.AluOpType.mult,
                                op1=mybir.AluOpType.add,
                            )
                        pw.append(p)

                # aggregation via dense per-tile adjacency count blocks
                for t_i in range(NT):
                    At = apool.tile([P, NT, P], fp8, name="At", tag="At")
                    nc.sync.dma_start(out=At[:], in_=ins_d[f"A_{g}"][t_i])
                    zt = psz.tile([P, H], f32, name="zt", tag="zt")
                    for j in range(NT // 2):
                        nc.tensor.matmul(
                            out=zt[:],
                            lhsT=At[:, 2 * j : 2 * j + 2, :],
                            rhs=y_sb[:, 2 * j : 2 * j + 2, :],
                            start=(j == 0),
                            stop=(j == NT // 2 - 1),
                            perf_mode=DR,
                        )
                    w, c = t_i // FINW, t_i % FINW
                    nc.vector.scalar_tensor_tensor(
                        out=e_sb[g][w][:, c, :],
                        in0=zt[:],
                        scalar=dinv_ga[g][:, t_i : t_i + 1],
                        in1=bias_ga[g][:],
                        op0=mybir.AluOpType.mult,
                        op1=mybir.AluOpType.add,
                    )
                    if g == "e" and c == FINW - 1:
                        # region-A wave: out = p_w + e_e
                        w0 = w * FINW
                        och = foa.tile([P, FINW, H], bft, name="ocha", tag="ocha")
                        nc.vector.tensor_tensor(
                            out=och[:], in0=pw[w][:], in1=e_sb["e"][w][:],
                            op=mybir.AluOpType.add,
                        )
                        nc.sync.dma_start(
                            out=out_d[:, w0 : w0 + FINW, :], in_=och[:]
                        )

    nc.compile()

    in_maps = [{k: v for k, v in cd.items()} for cd in core_data]
    global LAST_RESULT
    kw = {}
    if TRACE:
        kw = dict(trace=True, trace_cores=TRACE_CORES, stitch_traces=False)
    res = run_bass_kernel_spmd(nc, in_maps, list(range(8)), **kw)
    LAST_RESULT = res

    out = np.empty((B, S, D), np.float32)
    for b in range(B):
        sd = samp[b]
        ni = sd["ni"]
        tokB_valid = sd["tokB"][ni:]             # class-iii tokens
        boff = N + (ni - LI0 * P)                # their first output row
        for h in range(2):
            o = np.asarray(res.results[2 * b + h]["out"], dtype=np.float32)
            rows = o.transpose(1, 0, 2).reshape(C * P, H)
            out[b, sd["tokA"], h * H : (h + 1) * H] = rows[:N]
            out[b, tokB_valid, h * H : (h + 1) * H] = rows[boff : boff + len(tokB_valid)]
        out[b, sd["seg3_tok"], :] = t_full[b, sd["seg3_tok"], :]
    return out
